# revision 53
# baseline (speedup 1.0000x reference)
"""Trainium2 Bass kernel for nn_BonzSelfAttention.

Data-parallel over batch: B=8 batch elements -> 8 NeuronCores, one full
per-batch-element transformer block per core. No collectives.

Per-core math (x: [N=2048, D=768]):
  qT  = grouped_conv_q(x)^T               [D, N]   (PE, block-diag weights)
  xpT = x^T @ project_k                   [D, K]   (project first: N->K=256)
  k_proj = grouped_conv_k(xpT)            [K, D]   (natural layout)
  per head h (12 heads, dh=64):
    keysT_h[dh,K2] = k_proj[64*(h%4)+dh, 3*K2 + h//4]   (strided AP - free!)
    dotsT = keysT_h^T @ qT_h              [256, N] (psum, fp32)
    PT    = exp(dotsT/8)                  (ACT, no max-subtraction needed)
    ctxT_h|s_h = [V_h | ones]^T @ PT      [65, N]  (ones col -> softmax sums)
    after each head pair: 1/s via DRAM-broadcast + recip, scale ctxT tile
  out2 = ctxT^T @ w_out^T + b_out (ones-row bias trick), y = out2 + x (fp32)
  LN(y) via STT-accum mean + ACT Square/Sqrt + DVE tensor_scalar

Matmul operands are bf16 (fp32 PSUM accumulation); residual/LN stay fp32.
Measured end-to-end relative error ~3e-4.
"""
import sys

if "/opt/trn_rl_repo" not in sys.path:
    sys.path.insert(0, "/opt/trn_rl_repo")

from contextlib import ExitStack

import ml_dtypes
import numpy as np

import concourse.bass as bass
import concourse.bacc as bacc
import concourse.mybir as mybir
import concourse.tile as tile
from concourse.bass_utils import run_bass_kernel_spmd

FP = mybir.dt.float32
BF = mybir.dt.bfloat16
AF = mybir.ActivationFunctionType
ALU = mybir.AluOpType

B, N, D = 8, 2048, 768
K, H, G = 256, 12, 4
DH, GD = 64, 192
EPS = 1e-12
NCORES = 8


def build_program():
    nc = bacc.Bacc(None, target_bir_lowering=False)
    xt = nc.declare_dram_parameter("xt", [D, N], BF, isOutput=False)
    xa = nc.declare_dram_parameter("xa", [N, D], BF, isOutput=False)
    x_d = nc.declare_dram_parameter("x", [N, D], FP, isOutput=False)
    wqt = nc.declare_dram_parameter("wqt", [D, GD], BF, isOutput=False)  # [g*192+i, o]
    wkt = nc.declare_dram_parameter("wkt", [D, GD], BF, isOutput=False)
    pk_d = nc.declare_dram_parameter("pk", [N, K], BF, isOutput=False)
    wot = nc.declare_dram_parameter("wot", [D, D], BF, isOutput=False)  # w_out.T [d, c]
    bvec = nc.declare_dram_parameter("bvec", [1, D], BF, isOutput=False)
    out_d = nc.declare_dram_parameter("out", [N, D], FP, isOutput=True)
    sg_dram = nc.dram_tensor("sg_scratch", [H, N], FP)

    with tile.TileContext(nc) as tc, ExitStack() as top:
        persist = top.enter_context(tc.tile_pool(name="persist", bufs=1))
        ctxT = persist.tile([128, 6, N], BF)   # [d%128, d//128, n]
        # per-head softmax sums; head h lives at partition 32*(h%4), slot h//4
        # (engine SBUF APs may only start at partition 0/32/64/96)
        sg3 = persist.tile([128, 3, N], FP)

        ab = top.enter_context(tc.tile_pool(name="ab", bufs=1))
        qT = ab.tile([128, 6, N], BF)          # [d%128, d//128, n]
        kproj = ab.tile([128, 2, D], BF)       # [k%128, k//128, d]

        # ---------------- Phase A: convs + projection ----------------
        with ExitStack() as pha:
            ca = pha.enter_context(tc.tile_pool(name="constsA", bufs=1))
            wqg = ca.tile([128, 6, GD], BF)    # same 128-grid over d as xt
            wkg = ca.tile([128, 6, GD], BF)
            xpT = ca.tile([128, 6, K], BF)     # [d%128, d//128, k]

            xts = pha.enter_context(tc.tile_pool(name="xts", bufs=2))
            xs = pha.enter_context(tc.tile_pool(name="xs", bufs=3))
            pks = pha.enter_context(tc.tile_pool(name="pks", bufs=3))

            # contraction chunks per group on the global 128-grid of d
            def g_chunks(g):
                d0 = g * GD
                c = []
                while d0 < (g + 1) * GD:
                    t, p = d0 // 128, d0 % 128
                    sz = min(128 - p, (g + 1) * GD - d0)
                    c.append((t, p, sz, d0 - g * GD))  # tile, poff, size, local
                    d0 += sz
                return c

            with ExitStack() as phxp:
                xpps = phxp.enter_context(
                    tc.tile_pool(name="xpps", bufs=1, space="PSUM"))
                xpp = [xpps.tile([128, K], FP, tag=f"xp{t}", name=f"xpp{t}")
                       for t in range(6)]
                for ncn in range(16):
                    xc = xs.tile([128, D], BF, tag="xa")
                    nc.sync.dma_start(out=xc, in_=xa[ncn * 128:(ncn + 1) * 128, :])
                    pkc = pks.tile([128, K], BF, tag="pk")
                    nc.sync.dma_start(out=pkc, in_=pk_d[ncn * 128:(ncn + 1) * 128, :])
                    for t in range(6):
                        nc.tensor.matmul(
                            xpp[t],
                            lhsT=xc[:, t * 128:(t + 1) * 128],
                            rhs=pkc,
                            start=(ncn == 0), stop=(ncn == 15),
                        )
                for t in range(6):
                    nc.vector.tensor_copy(xpT[:, t, :], xpp[t])
            # weight loads are slow-issue (768-descriptor rearranges): emit them
            # after the x/pk chunk DMAs so they don't delay the first matmuls
            nc.sync.dma_start(out=wqg, in_=wqt.rearrange("(t p) o -> p t o", p=128))
            nc.sync.dma_start(out=wkg, in_=wkt.rearrange("(t p) o -> p t o", p=128))
            qps = pha.enter_context(tc.tile_pool(name="qps", bufs=4, space="PSUM"))

            # qT grouped conv, streaming xt column blocks
            for nh in range(2):
                for ns in range(2):
                    n0 = nh * 1024 + ns * 512
                    xtb = xts.tile([128, 6, 512], BF, tag="xtb")
                    nc.sync.dma_start(
                        out=xtb,
                        in_=xt.rearrange("(t p) n -> p t n", p=128)[:, :, n0:n0 + 512])
                    for g in range(G):
                        for (ot, op_, osz, olo) in g_chunks(g):
                            ps = qps.tile([128, 512], FP, tag="qps")
                            first = True
                            for (it, ip, isz, ilo) in g_chunks(g):
                                nc.tensor.matmul(
                                    ps[:osz, :],
                                    lhsT=wqg[ip:ip + isz, it, olo:olo + osz],
                                    rhs=xtb[ip:ip + isz, it, :],
                                    start=first, stop=not first,
                                )
                                first = False
                            if (g + ot) % 2 == 0:
                                nc.vector.tensor_copy(
                                    qT[op_:op_ + osz, ot, n0:n0 + 512],
                                    ps[:osz, :])
                            else:
                                nc.scalar.copy(
                                    qT[op_:op_ + osz, ot, n0:n0 + 512],
                                    ps[:osz, :])

            # k_proj grouped conv from xpT (small: K=256 rows)
            with ExitStack() as phkp:
                kpps = phkp.enter_context(
                    tc.tile_pool(name="kpps", bufs=2, space="PSUM"))
                for kc in range(2):
                    for g in range(G):
                        ps = kpps.tile([128, GD], FP, tag="kp")
                        first = True
                        for (it, ip, isz, ilo) in g_chunks(g):
                            nc.tensor.matmul(
                                ps,
                                lhsT=xpT[ip:ip + isz, it, kc * 128:(kc + 1) * 128],
                                rhs=wkg[ip:ip + isz, it, :],
                                start=first, stop=not first,
                            )
                            first = False
                        nc.vector.tensor_copy(
                            kproj[:, kc, g * GD:(g + 1) * GD], ps)

        # ---------------- Phase B: attention (+ inline normalize) ----------
        # one rotating 4-slot psum pool shared by attention AND the
        # out-projection, so phase C's accumulations can begin while the last
        # head pair is still normalizing (no pool-close barrier)
        dps = top.enter_context(tc.tile_pool(name="dps", bufs=4, space="PSUM"))
        with ExitStack() as phb:
            bvau = phb.enter_context(tc.tile_pool(name="vaug", bufs=1))
            # padded to 128 columns so LDWEIGHTS gets fast-weight-load
            vaug = bvau.tile([128, 2 * H, 128], BF)
            nc.gpsimd.memset(vaug[:, :, DH:128], 0.0)
            nc.gpsimd.memset(vaug[:, :, DH:DH + 1], 1.0)
            for h in range(H):
                for kc in range(2):
                    nc.gpsimd.tensor_copy(
                        vaug[:, 2 * h + kc, 0:DH],
                        kproj[:, kc, h * DH:(h + 1) * DH])

            pts = phb.enter_context(tc.tile_pool(name="pts", bufs=2))
            reps = phb.enter_context(tc.tile_pool(name="reps", bufs=2))
            rreps = phb.enter_context(tc.tile_pool(name="rreps", bufs=2))

            for hp in range(H // 2):
                # heads (2*hp, 2*hp+1): their keys/q live at partition offsets
                # 0/64, so adjacent dots matmuls land in disjoint PE row-groups
                # and execute concurrently (row-tiling).
                h0 = 2 * hp
                pt2 = [pts.tile([128, 2, N], BF, tag="pt", name=f"pt{j}")
                       for j in range(2)]
                for nb in range(2):
                    for k2c in range(2):
                        dp2 = [dps.tile([128, N // 2], FP, tag="ps8",
                                        name=f"dp{j}") for j in range(2)]
                        for ns2 in range(2):
                            n0 = nb * 1024 + ns2 * 512
                            for j in range(2):
                                h = h0 + j
                                base = h // 4 + 384 * k2c
                                nc.tensor.matmul(
                                    dp2[j][:, ns2 * 512:(ns2 + 1) * 512],
                                    lhsT=kproj[64 * ((h % 4) % 2):
                                               64 * ((h % 4) % 2) + 64,
                                               (h % 4) // 2,
                                               base:base + 382:3],
                                    rhs=qT[64 * (h % 2):64 * (h % 2) + 64,
                                           h // 2, n0:n0 + 512],
                                    start=True, stop=True,
                                )
                        for j in range(2):
                            nc.scalar.activation(
                                pt2[j][:, k2c, nb * 1024:(nb + 1) * 1024],
                                dp2[j], AF.Exp, scale=0.125)
                    for j in range(2):
                        h = h0 + j
                        cp = dps.tile([128, N // 2], FP, tag="ps8", name="cp")
                        for ns2 in range(2):
                            for k2c in range(2):
                                n0 = nb * 1024 + ns2 * 512
                                nc.tensor.matmul(
                                    cp[:, ns2 * 512:(ns2 + 1) * 512],
                                    lhsT=vaug[:, 2 * h + k2c, :],
                                    rhs=pt2[j][:, k2c, n0:n0 + 512],
                                    start=(k2c == 0), stop=(k2c == 1),
                                )
                        nsl = slice(nb * 1024, (nb + 1) * 1024)
                        c_off, c_t = 64 * (h % 2), h // 2
                        sgp = 32 * (h % 4)
                        nc.vector.tensor_copy(
                            ctxT[c_off:c_off + 64, c_t, nsl], cp[0:DH, :])
                        if h % 2 == 0:
                            nc.scalar.copy(sg3[sgp:sgp + 1, h // 4, nsl],
                                           cp[DH:DH + 1, :])
                        else:
                            nc.vector.tensor_copy(
                                sg3[sgp:sgp + 1, h // 4, nsl],
                                cp[DH:DH + 1, :])
                        nc.sync.dma_start(out=sg_dram[h:h + 1, nsl],
                                          in_=sg3[sgp:sgp + 1, h // 4, nsl])

                # normalize ctxT tile t = hp per n-half, pipelined: the nb0
                # chain (bcast->recip->mul) overlaps nb1's attention work so
                # only ~half the chain remains after the last s-copy
                t = hp
                for nb in range(2):
                    nsl = slice(nb * 1024, (nb + 1) * 1024)
                    rep = reps.tile([128, N // 2], FP, tag="rep")
                    for j, hh in enumerate((2 * t, 2 * t + 1)):
                        row = sg_dram[hh:hh + 1, nsl]
                        bc = bass.AP(tensor=row.tensor, offset=row.offset,
                                     ap=[[0, 64]] + list(row.ap)[1:])
                        nc.sync.dma_start(out=rep[64 * j:64 * j + 64, :], in_=bc)
                    rrep = rreps.tile([128, N // 2], FP, tag="rrep")
                    nc.vector.reciprocal_approx_fast(rrep, rep)
                    if t >= 4:
                        # last tiles gate the out-projection: faster DVE path
                        nc.vector.tensor_mul(ctxT[:, t, nsl], ctxT[:, t, nsl],
                                             rrep)
                    else:
                        nc.gpsimd.tensor_mul(ctxT[:, t, nsl], ctxT[:, t, nsl],
                                             rrep)

        # ---------------- Phase C: out proj + residual + LN ----------------
        with ExitStack() as phc:
            cc = phc.enter_context(tc.tile_pool(name="constsC", bufs=1))
            wos = cc.tile([128, 6, D], BF)
            nc.sync.dma_start(out=wos, in_=wot.rearrange("(t p) c -> p t c", p=128))
            bias = cc.tile([1, D], BF)
            nc.sync.dma_start(out=bias, in_=bvec[:, :])
            ones1 = cc.tile([1, 128], BF)
            nc.vector.memset(ones1, 1.0)
            epsc = cc.tile([128, 1], FP)
            nc.vector.memset(epsc, EPS)

            xs2 = phc.enter_context(tc.tile_pool(name="xs2", bufs=4))
            sqs = phc.enter_context(tc.tile_pool(name="sqs", bufs=3))
            ys = phc.enter_context(tc.tile_pool(name="ys", bufs=4))
            ofs = phc.enter_context(tc.tile_pool(name="ofs", bufs=4))
            sts = phc.enter_context(tc.tile_pool(name="sts", bufs=8))

            for ncn in range(16):
                xc = xs2.tile([128, D], FP, tag="xc")
                nc.sync.dma_start(out=xc, in_=x_d[ncn * 128:(ncn + 1) * 128, :])
                opt_ = dps.tile([128, N // 2], FP, tag="ps8", name="op")
                op = opt_[:, 0:D]
                for (c0, csz) in ((0, 512), (512, 256)):
                    for t in range(6):
                        nc.tensor.matmul(
                            op[:, c0:c0 + csz],
                            lhsT=ctxT[:, t, ncn * 128:(ncn + 1) * 128],
                            rhs=wos[:, t, c0:c0 + csz],
                            start=(t == 0), stop=False,
                        )
                    nc.tensor.matmul(
                        op[:, c0:c0 + csz],
                        lhsT=ones1, rhs=bias[:, c0:c0 + csz],
                        start=False, stop=True,
                    )
                y = ys.tile([128, D], FP, tag="y")
                ysum = sts.tile([128, 1], FP, tag="ysum")
                nc.vector.scalar_tensor_tensor(
                    out=y, in0=op, scalar=1.0, in1=xc,
                    op0=ALU.mult, op1=ALU.add, accum_out=ysum)
                negmu = sts.tile([128, 1], FP, tag="negmu")
                nc.vector.tensor_scalar_mul(negmu, ysum, -1.0 / D)
                sq = sqs.tile([128, D], FP, tag="sq")
                ssq = sts.tile([128, 1], FP, tag="ssq")
                nc.scalar.activation(sq, y, AF.Square, bias=negmu, scale=1.0,
                                     accum_out=ssq)
                std = sts.tile([128, 1], FP, tag="std")
                nc.scalar.activation(std, ssq, AF.Sqrt, bias=epsc, scale=1.0 / D)
                rstd = sts.tile([128, 1], FP, tag="rstd")
                nc.vector.reciprocal(rstd, std)
                nmr = sts.tile([128, 1], FP, tag="nmr")
                nc.vector.tensor_mul(nmr, negmu, rstd)
                of = ofs.tile([128, D], FP, tag="of")
                nc.vector.tensor_scalar(
                    out=of, in0=y, scalar1=rstd, scalar2=nmr,
                    op0=ALU.mult, op1=ALU.add)
                nc.sync.dma_start(out=out_d[ncn * 128:(ncn + 1) * 128, :], in_=of)

    return nc


_NC_CACHE = None


def _get_nc():
    global _NC_CACHE
    if _NC_CACHE is None:
        nc = build_program()
        if not nc.is_finalized():
            nc.finalize()   # runs Bacc.compile(): reg alloc, wait splitting
        _NC_CACHE = nc
    return _NC_CACHE


def _bf(a):
    return np.ascontiguousarray(a.astype(ml_dtypes.bfloat16))


def make_in_maps(inputs):
    x = np.asarray(inputs["input_embedding"], np.float32)
    wq = np.asarray(inputs["wq"], np.float32)
    wk = np.asarray(inputs["wk"], np.float32)
    pk = np.asarray(inputs["project_k"], np.float32)
    w_out = np.asarray(inputs["w_out"], np.float32)
    b_out = np.asarray(inputs["b_out"], np.float32)

    wqt = _bf(np.transpose(wq, (0, 2, 1)).reshape(D, GD))  # [g*192+i, o]
    wkt = _bf(np.transpose(wk, (0, 2, 1)).reshape(D, GD))
    wot = _bf(w_out.T)
    bvec = _bf(b_out.reshape(1, D))
    pk_bf = _bf(pk)

    in_maps = []
    for c in range(NCORES):
        xc = np.ascontiguousarray(x[c])
        in_maps.append({
            "xt": _bf(xc.T), "xa": _bf(xc), "x": xc,
            "wqt": wqt, "wkt": wkt, "pk": pk_bf, "wot": wot, "bvec": bvec,
        })
    return in_maps


def kernel(**inputs):
    gamma = np.asarray(inputs["gamma"], np.float32)
    beta = np.asarray(inputs["beta"], np.float32)
    nc = _get_nc()
    in_maps = make_in_maps(inputs)
    res = run_bass_kernel_spmd(nc, in_maps, list(range(NCORES)))
    outs = np.stack([np.asarray(res.results[c]["out"]) for c in range(NCORES)])

    # gamma/beta are affine post-LN params; apply on host only if non-trivial
    if not (np.all(gamma == 1.0) and np.all(beta == 0.0)):
        outs = outs * gamma[None, None, :] + beta[None, None, :]
    return outs.astype(np.float32)


if __name__ == "__main__":
    nc = build_program()
    print("program built ok")


# revision 55
# speedup vs baseline: 1.0397x; 1.0397x over previous
"""Trainium2 Bass kernel for nn_BonzSelfAttention.

Data-parallel over batch: B=8 batch elements -> 8 NeuronCores, one full
per-batch-element transformer block per core. No collectives.

Per-core math (x: [N=2048, D=768]):
  qT  = grouped_conv_q(x)^T               [D, N]   (PE, block-diag weights)
  xpT = x^T @ project_k                   [D, K]   (project first: N->K=256)
  k_proj = grouped_conv_k(xpT)            [K, D]   (natural layout)
  per head h (12 heads, dh=64):
    keysT_h[dh,K2] = k_proj[64*(h%4)+dh, 3*K2 + h//4]   (strided AP - free!)
    dotsT = keysT_h^T @ qT_h              [256, N] (psum, fp32)
    PT    = exp(dotsT/8)                  (ACT, no max-subtraction needed)
    ctxT_h|s_h = [V_h | ones]^T @ PT      [65, N]  (ones col -> softmax sums)
    after each head pair: 1/s via DRAM-broadcast + recip, scale ctxT tile
  out2 = ctxT^T @ w_out^T + b_out (ones-row bias trick), y = out2 + x (fp32)
  LN(y) via STT-accum mean + ACT Square/Sqrt + DVE tensor_scalar

Matmul operands are bf16 (fp32 PSUM accumulation); residual/LN stay fp32.
Measured end-to-end relative error ~3e-4.
"""
import sys

if "/opt/trn_rl_repo" not in sys.path:
    sys.path.insert(0, "/opt/trn_rl_repo")

from contextlib import ExitStack

import ml_dtypes
import numpy as np

import concourse.bass as bass
import concourse.bacc as bacc
import concourse.mybir as mybir
import concourse.tile as tile
from concourse.bass_utils import run_bass_kernel_spmd

FP = mybir.dt.float32
BF = mybir.dt.bfloat16
AF = mybir.ActivationFunctionType
ALU = mybir.AluOpType

B, N, D = 8, 2048, 768
K, H, G = 256, 12, 4
DH, GD = 64, 192
EPS = 1e-12
NCORES = 8


def build_program():
    nc = bacc.Bacc(None, target_bir_lowering=False)
    xt = nc.declare_dram_parameter("xt", [D, N], BF, isOutput=False)
    xa = nc.declare_dram_parameter("xa", [N, D], BF, isOutput=False)
    x_d = nc.declare_dram_parameter("x", [N, D], FP, isOutput=False)
    wqt = nc.declare_dram_parameter("wqt", [D, GD], BF, isOutput=False)  # [g*192+i, o]
    wkt = nc.declare_dram_parameter("wkt", [D, GD], BF, isOutput=False)
    pk_d = nc.declare_dram_parameter("pk", [N, K], BF, isOutput=False)
    wot = nc.declare_dram_parameter("wot", [D, D], BF, isOutput=False)  # w_out.T [d, c]
    bvec = nc.declare_dram_parameter("bvec", [1, D], BF, isOutput=False)
    out_d = nc.declare_dram_parameter("out", [N, D], FP, isOutput=True)
    sg_dram = nc.dram_tensor("sg_scratch", [H, N], FP)

    with tile.TileContext(nc) as tc, ExitStack() as top:
        persist = top.enter_context(tc.tile_pool(name="persist", bufs=1))
        ctxT = persist.tile([128, 6, N], BF)   # [d%128, d//128, n]
        # per-head softmax sums; head h lives at partition 32*(h%4), slot h//4
        # (engine SBUF APs may only start at partition 0/32/64/96)
        sg3 = persist.tile([128, 3, N], FP)

        ab = top.enter_context(tc.tile_pool(name="ab", bufs=1))
        qT = ab.tile([128, 6, N], BF)          # [d%128, d//128, n]
        kproj = ab.tile([128, 2, D], BF)       # [k%128, k//128, d]

        # ---------------- Phase A: convs + projection ----------------
        with ExitStack() as pha:
            ca = pha.enter_context(tc.tile_pool(name="constsA", bufs=1))
            wqg = ca.tile([128, 6, GD], BF)    # same 128-grid over d as xt
            wkg = ca.tile([128, 6, GD], BF)
            xpT = ca.tile([128, 6, K], BF)     # [d%128, d//128, k]

            xts = pha.enter_context(tc.tile_pool(name="xts", bufs=2))
            xs = pha.enter_context(tc.tile_pool(name="xs", bufs=3))
            pks = pha.enter_context(tc.tile_pool(name="pks", bufs=3))

            # contraction chunks per group on the global 128-grid of d
            def g_chunks(g):
                d0 = g * GD
                c = []
                while d0 < (g + 1) * GD:
                    t, p = d0 // 128, d0 % 128
                    sz = min(128 - p, (g + 1) * GD - d0)
                    c.append((t, p, sz, d0 - g * GD))  # tile, poff, size, local
                    d0 += sz
                return c

            with ExitStack() as phxp:
                xpps = phxp.enter_context(
                    tc.tile_pool(name="xpps", bufs=1, space="PSUM"))
                xpp = [xpps.tile([128, K], FP, tag=f"xp{t}", name=f"xpp{t}")
                       for t in range(6)]
                for ncn in range(16):
                    xc = xs.tile([128, D], BF, tag="xa")
                    nc.sync.dma_start(out=xc, in_=xa[ncn * 128:(ncn + 1) * 128, :])
                    pkc = pks.tile([128, K], BF, tag="pk")
                    nc.sync.dma_start(out=pkc, in_=pk_d[ncn * 128:(ncn + 1) * 128, :])
                    for t in range(6):
                        nc.tensor.matmul(
                            xpp[t],
                            lhsT=xc[:, t * 128:(t + 1) * 128],
                            rhs=pkc,
                            start=(ncn == 0), stop=(ncn == 15),
                        )
                for t in range(6):
                    nc.vector.tensor_copy(xpT[:, t, :], xpp[t])
            # weight loads are slow-issue (768-descriptor rearranges): emit them
            # after the x/pk chunk DMAs so they don't delay the first matmuls
            nc.sync.dma_start(out=wqg, in_=wqt.rearrange("(t p) o -> p t o", p=128))
            nc.sync.dma_start(out=wkg, in_=wkt.rearrange("(t p) o -> p t o", p=128))
            qps = pha.enter_context(tc.tile_pool(name="qps", bufs=4, space="PSUM"))

            # qT grouped conv, streaming xt column blocks
            for nh in range(2):
                for ns in range(2):
                    n0 = nh * 1024 + ns * 512
                    xtb = xts.tile([128, 6, 512], BF, tag="xtb")
                    nc.sync.dma_start(
                        out=xtb,
                        in_=xt.rearrange("(t p) n -> p t n", p=128)[:, :, n0:n0 + 512])
                    for g in range(G):
                        for (ot, op_, osz, olo) in g_chunks(g):
                            ps = qps.tile([128, 512], FP, tag="qps")
                            first = True
                            for (it, ip, isz, ilo) in g_chunks(g):
                                nc.tensor.matmul(
                                    ps[:osz, :],
                                    lhsT=wqg[ip:ip + isz, it, olo:olo + osz],
                                    rhs=xtb[ip:ip + isz, it, :],
                                    start=first, stop=not first,
                                )
                                first = False
                            if (g + ot) % 2 == 0:
                                nc.vector.tensor_copy(
                                    qT[op_:op_ + osz, ot, n0:n0 + 512],
                                    ps[:osz, :])
                            else:
                                nc.scalar.copy(
                                    qT[op_:op_ + osz, ot, n0:n0 + 512],
                                    ps[:osz, :])

            # k_proj grouped conv from xpT (small: K=256 rows)
            with ExitStack() as phkp:
                kpps = phkp.enter_context(
                    tc.tile_pool(name="kpps", bufs=2, space="PSUM"))
                for kc in range(2):
                    for g in range(G):
                        ps = kpps.tile([128, GD], FP, tag="kp")
                        first = True
                        for (it, ip, isz, ilo) in g_chunks(g):
                            nc.tensor.matmul(
                                ps,
                                lhsT=xpT[ip:ip + isz, it, kc * 128:(kc + 1) * 128],
                                rhs=wkg[ip:ip + isz, it, :],
                                start=first, stop=not first,
                            )
                            first = False
                        nc.vector.tensor_copy(
                            kproj[:, kc, g * GD:(g + 1) * GD], ps)

        # ---------------- Phase B: attention (+ inline normalize) ----------
        # one rotating 4-slot psum pool shared by attention AND the
        # out-projection, so phase C's accumulations can begin while the last
        # head pair is still normalizing (no pool-close barrier)
        dps = top.enter_context(tc.tile_pool(name="dps", bufs=4, space="PSUM"))
        with ExitStack() as phb:
            bvau = phb.enter_context(tc.tile_pool(name="vaug", bufs=1))
            # padded to 128 columns so LDWEIGHTS gets fast-weight-load
            vaug = bvau.tile([128, 2 * H, 128], BF)
            nc.gpsimd.memset(vaug[:, :, DH:128], 0.0)
            nc.gpsimd.memset(vaug[:, :, DH:DH + 1], 1.0)
            for h in range(H):
                for kc in range(2):
                    nc.gpsimd.tensor_copy(
                        vaug[:, 2 * h + kc, 0:DH],
                        kproj[:, kc, h * DH:(h + 1) * DH])

            pts = phb.enter_context(tc.tile_pool(name="pts", bufs=3))
            reps = phb.enter_context(tc.tile_pool(name="reps", bufs=2))
            rreps = phb.enter_context(tc.tile_pool(name="rreps", bufs=2))

            for hp in range(H // 2):
                # heads (2*hp, 2*hp+1): their keys/q live at partition offsets
                # 0/64, so adjacent dots matmuls land in disjoint PE row-groups
                # and execute concurrently (row-tiling).
                h0 = 2 * hp
                pt2 = [pts.tile([128, 2, N], BF, tag="pt", name=f"pt{j}")
                       for j in range(2)]
                for nb in range(2):
                    for k2c in range(2):
                        dp2 = [dps.tile([128, N // 2], FP, tag="ps8",
                                        name=f"dp{j}") for j in range(2)]
                        for ns2 in range(2):
                            n0 = nb * 1024 + ns2 * 512
                            for j in range(2):
                                h = h0 + j
                                base = h // 4 + 384 * k2c
                                nc.tensor.matmul(
                                    dp2[j][:, ns2 * 512:(ns2 + 1) * 512],
                                    lhsT=kproj[64 * ((h % 4) % 2):
                                               64 * ((h % 4) % 2) + 64,
                                               (h % 4) // 2,
                                               base:base + 382:3],
                                    rhs=qT[64 * (h % 2):64 * (h % 2) + 64,
                                           h // 2, n0:n0 + 512],
                                    start=True, stop=True,
                                )
                        for j in range(2):
                            nc.scalar.activation(
                                pt2[j][:, k2c, nb * 1024:(nb + 1) * 1024],
                                dp2[j], AF.Exp, scale=0.125)
                    for j in range(2):
                        h = h0 + j
                        cp = dps.tile([128, N // 2], FP, tag="ps8", name="cp")
                        for ns2 in range(2):
                            for k2c in range(2):
                                n0 = nb * 1024 + ns2 * 512
                                nc.tensor.matmul(
                                    cp[:, ns2 * 512:(ns2 + 1) * 512],
                                    lhsT=vaug[:, 2 * h + k2c, :],
                                    rhs=pt2[j][:, k2c, n0:n0 + 512],
                                    start=(k2c == 0), stop=(k2c == 1),
                                )
                        nsl = slice(nb * 1024, (nb + 1) * 1024)
                        c_off, c_t = 64 * (h % 2), h // 2
                        sgp = 32 * (h % 4)
                        nc.vector.tensor_copy(
                            ctxT[c_off:c_off + 64, c_t, nsl], cp[0:DH, :])
                        if h % 2 == 0:
                            nc.scalar.copy(sg3[sgp:sgp + 1, h // 4, nsl],
                                           cp[DH:DH + 1, :])
                        else:
                            nc.vector.tensor_copy(
                                sg3[sgp:sgp + 1, h // 4, nsl],
                                cp[DH:DH + 1, :])
                for h in (h0, h0 + 1):
                    sgp = 32 * (h % 4)
                    nc.sync.dma_start(out=sg_dram[h:h + 1, :],
                                      in_=sg3[sgp:sgp + 1, h // 4, :])

                h = h0 + 1
                if h % 2 == 1:
                    # normalize ctxT tile t = h//2 (heads h-1, h) right away so
                    # the out-projection never stalls on softmax sums
                    t = h // 2
                    rep = reps.tile([128, N], FP, tag="rep")
                    for j, hh in enumerate((2 * t, 2 * t + 1)):
                        row = sg_dram[hh:hh + 1, :]
                        bc = bass.AP(tensor=row.tensor, offset=row.offset,
                                     ap=[[0, 64]] + list(row.ap)[1:])
                        nc.sync.dma_start(out=rep[64 * j:64 * j + 64, :], in_=bc)
                    rrep = rreps.tile([128, N], FP, tag="rrep")
                    nc.vector.reciprocal_approx_fast(rrep, rep)
                    if t == 5:
                        # last tile gates the out-projection: use the faster DVE
                        nc.vector.tensor_mul(ctxT[:, t, :], ctxT[:, t, :], rrep)
                    else:
                        nc.gpsimd.tensor_mul(ctxT[:, t, :], ctxT[:, t, :], rrep)

        # ---------------- Phase C: out proj + residual + LN ----------------
        with ExitStack() as phc:
            cc = phc.enter_context(tc.tile_pool(name="constsC", bufs=1))
            wos = cc.tile([128, 6, D], BF)
            nc.sync.dma_start(out=wos, in_=wot.rearrange("(t p) c -> p t c", p=128))
            bias = cc.tile([1, D], BF)
            nc.sync.dma_start(out=bias, in_=bvec[:, :])
            ones1 = cc.tile([1, 128], BF)
            nc.vector.memset(ones1, 1.0)
            epsc = cc.tile([128, 1], FP)
            nc.vector.memset(epsc, EPS)

            xs2 = phc.enter_context(tc.tile_pool(name="xs2", bufs=4))
            sqs = phc.enter_context(tc.tile_pool(name="sqs", bufs=3))
            ys = phc.enter_context(tc.tile_pool(name="ys", bufs=4))
            ofs = phc.enter_context(tc.tile_pool(name="ofs", bufs=4))
            sts = phc.enter_context(tc.tile_pool(name="sts", bufs=8))

            for ncn in range(16):
                xc = xs2.tile([128, D], FP, tag="xc")
                nc.sync.dma_start(out=xc, in_=x_d[ncn * 128:(ncn + 1) * 128, :])
                opt_ = dps.tile([128, N // 2], FP, tag="ps8", name="op")
                op = opt_[:, 0:D]
                for (c0, csz) in ((0, 512), (512, 256)):
                    for t in range(6):
                        nc.tensor.matmul(
                            op[:, c0:c0 + csz],
                            lhsT=ctxT[:, t, ncn * 128:(ncn + 1) * 128],
                            rhs=wos[:, t, c0:c0 + csz],
                            start=(t == 0), stop=False,
                        )
                    nc.tensor.matmul(
                        op[:, c0:c0 + csz],
                        lhsT=ones1, rhs=bias[:, c0:c0 + csz],
                        start=False, stop=True,
                    )
                y = ys.tile([128, D], FP, tag="y")
                ysum = sts.tile([128, 1], FP, tag="ysum")
                nc.vector.scalar_tensor_tensor(
                    out=y, in0=op, scalar=1.0, in1=xc,
                    op0=ALU.mult, op1=ALU.add, accum_out=ysum)
                negmu = sts.tile([128, 1], FP, tag="negmu")
                nc.vector.tensor_scalar_mul(negmu, ysum, -1.0 / D)
                sq = sqs.tile([128, D], FP, tag="sq")
                ssq = sts.tile([128, 1], FP, tag="ssq")
                nc.scalar.activation(sq, y, AF.Square, bias=negmu, scale=1.0,
                                     accum_out=ssq)
                std = sts.tile([128, 1], FP, tag="std")
                nc.scalar.activation(std, ssq, AF.Sqrt, bias=epsc, scale=1.0 / D)
                rstd = sts.tile([128, 1], FP, tag="rstd")
                nc.vector.reciprocal(rstd, std)
                nmr = sts.tile([128, 1], FP, tag="nmr")
                nc.vector.tensor_mul(nmr, negmu, rstd)
                of = ofs.tile([128, D], FP, tag="of")
                nc.vector.tensor_scalar(
                    out=of, in0=y, scalar1=rstd, scalar2=nmr,
                    op0=ALU.mult, op1=ALU.add)
                nc.sync.dma_start(out=out_d[ncn * 128:(ncn + 1) * 128, :], in_=of)

    return nc


_NC_CACHE = None


def _get_nc():
    global _NC_CACHE
    if _NC_CACHE is None:
        nc = build_program()
        if not nc.is_finalized():
            nc.finalize()   # runs Bacc.compile(): reg alloc, wait splitting
        _NC_CACHE = nc
    return _NC_CACHE


def _bf(a):
    return np.ascontiguousarray(a.astype(ml_dtypes.bfloat16))


def make_in_maps(inputs):
    x = np.asarray(inputs["input_embedding"], np.float32)
    wq = np.asarray(inputs["wq"], np.float32)
    wk = np.asarray(inputs["wk"], np.float32)
    pk = np.asarray(inputs["project_k"], np.float32)
    w_out = np.asarray(inputs["w_out"], np.float32)
    b_out = np.asarray(inputs["b_out"], np.float32)

    wqt = _bf(np.transpose(wq, (0, 2, 1)).reshape(D, GD))  # [g*192+i, o]
    wkt = _bf(np.transpose(wk, (0, 2, 1)).reshape(D, GD))
    wot = _bf(w_out.T)
    bvec = _bf(b_out.reshape(1, D))
    pk_bf = _bf(pk)

    in_maps = []
    for c in range(NCORES):
        xc = np.ascontiguousarray(x[c])
        in_maps.append({
            "xt": _bf(xc.T), "xa": _bf(xc), "x": xc,
            "wqt": wqt, "wkt": wkt, "pk": pk_bf, "wot": wot, "bvec": bvec,
        })
    return in_maps


def kernel(**inputs):
    gamma = np.asarray(inputs["gamma"], np.float32)
    beta = np.asarray(inputs["beta"], np.float32)
    nc = _get_nc()
    in_maps = make_in_maps(inputs)
    res = run_bass_kernel_spmd(nc, in_maps, list(range(NCORES)))
    outs = np.stack([np.asarray(res.results[c]["out"]) for c in range(NCORES)])

    # gamma/beta are affine post-LN params; apply on host only if non-trivial
    if not (np.all(gamma == 1.0) and np.all(beta == 0.0)):
        outs = outs * gamma[None, None, :] + beta[None, None, :]
    return outs.astype(np.float32)


if __name__ == "__main__":
    nc = build_program()
    print("program built ok")
